# revision 9
# baseline (speedup 1.0000x reference)
"""Trainium2 Bass kernel for nn_JointCatPoseModel (moe_routing).

Computation (reference):
    y0   = x @ Wfc + bfc                         [B, C]
    ybin = einsum('bn,ckn->bck', x, Wbin) + bbin [B, C, K]
    ind  = argmax_k ybin                         [B, C]
    yres = (einsum('bn,cknd->bckd', x, Wres) + bres)[b, c, ind, d]
    y1   = (centers[ind] + yres).transpose(0, 2, 1)   [B, D, C]

Strategy: data-parallel over batch across 8 NeuronCores (2048 rows each,
weights replicated, no collectives).  Per core, one fused GEMM
x_tile[128, 1024] @ W_cat[1024, 3084] where W_cat = [Wfc | Wbin(ck) |
Wres(dck)], computed in 7 PSUM column groups with the contraction split
into 8 chunks of 128.  Biases (incl. centers folded into the Wres bias)
are injected by a K=1 ones-vector matmul that initializes each PSUM
accumulation group.  The argmax/one-hot selection runs on the vector
engine: per-class reduce_max -> is_ge mask -> masked multiply + per-class
reduce_sum, writing the [d][c]-ordered output directly.
"""

import os

import numpy as np

B, N0, C, K, D = 16384, 1024, 12, 64, 3
NCORES = 8
BSH = B // NCORES          # 2048 rows per core
P = 128                    # partitions / batch tile rows
NTILES = BSH // P          # 16 batch tiles per core
KC = N0 // P               # 8 contraction chunks
CK = C * K                 # 768 bin columns
RES = C * K * D            # 2304 residual columns
NCOL = C + CK + RES        # 3084 fused GEMM columns
BIN_OFF = C                # ybin columns start
RES_OFF = C + CK           # yres columns start

# PSUM column groups: (start, size, needs_bias_matmul). Aligned so that the
# (possibly zero-bias) y0+ybin region and the always-biased yres region do
# not share a group.
_GROUPS_HEAD = [(0, 512), (512, 268)]
_GROUPS_RES = [(780, 512), (1292, 512), (1804, 512), (2316, 512), (2828, 256)]

_CACHE = {}


def _patch_tail_drain():
    """Cap sync waits at one per instruction.

    The walrus build in this container rejects instructions carrying more
    than one sync wait ("Too many sync wait commands"), but Tile's
    scheduler freely attaches several.  Semantically, waits attached to an
    instruction execute before it; moving all but one wait onto preceding
    same-engine NOPs is equivalent (the engine blocks on the NOPs first).
    Also splits the tail drain, which Tile builds outside the commit path.
    """
    import concourse.mybir as mybir
    from concourse.tile import ScopedClock, TileContext

    if getattr(TileContext, "_drain_split_patched", False):
        return

    _orig_commit = TileContext._commit_instruction

    def _commit_instruction(self, inst, lazy_reg_writes=True):
        si = inst.sync_info
        if si is not None and len(si.on_wait) > 1:
            waits = list(si.on_wait)
            inst.sync_info = mybir.SyncInfo(
                on_wait=waits[-1:], on_update=list(si.on_update)
            )
            for w in waits[:-1]:
                nop = mybir.InstNoOp(
                    name=self.nc.get_next_instruction_name(),
                    text_hint="wait_split",
                    bass_nofuse=True,
                    engine=inst.engine,
                    sync_info=mybir.SyncInfo(on_wait=[w], on_update=[]),
                )
                _orig_commit(self, nop, lazy_reg_writes=False)
        return _orig_commit(self, inst, lazy_reg_writes=lazy_reg_writes)

    def _drain_and_barrier(self, tick_clock, wait_clock):
        nc = self.nc
        drain_inst = nc.sync.drain()
        wait_clock.add_sem_waits(
            drain_inst.ins, ScopedClock({None: tick_clock.global_clock})
        )
        si = drain_inst.ins.sync_info
        waits = list(si.on_wait) if si else []
        if len(waits) > 1:
            drain_inst.ins.sync_info = mybir.SyncInfo(
                on_wait=waits[:1], on_update=list(si.on_update)
            )
            for w in waits[1:]:
                d2 = nc.sync.drain()
                d2.ins.sync_info = mybir.SyncInfo(on_wait=[w], on_update=[])
        nc.all_engine_barrier()
        popped = nc._tile_sem_poison_stack.pop()
        assert popped is self._sem_poison
        nc.clear_and_free_semaphores(list(self.sems.allocated().values()))
        nc.all_engine_barrier()

    TileContext._commit_instruction = _commit_instruction
    TileContext._drain_and_barrier = _drain_and_barrier
    TileContext._drain_split_patched = True


def _build(bias_head):
    """Build the per-core Bass module.  bias_head: emit bias matmuls for the
    y0/ybin groups too (needed only when bfc/bbin are nonzero)."""
    import concourse.bass as bass
    import concourse.mybir as mybir
    import concourse.tile as tile

    _patch_tail_drain()
    f32 = mybir.dt.float32
    X = mybir.AxisListType.X

    nc = bass.Bass(trn_type="TRN2")
    # x arrives pre-tiled on host: xP[t2, p, (bt2 kc b)] = x[t2*256 + bt2*128 + b,
    # kc*128 + p] so each load is [128 partitions x 8KB contiguous].
    xP = nc.dram_tensor("xP", [NTILES // 2, P, 2 * KC * P], f32, kind="ExternalInput")
    W = nc.dram_tensor("W", [N0, NCOL], f32, kind="ExternalInput")
    brow = nc.dram_tensor("brow", [1, NCOL], f32, kind="ExternalInput")
    y0o = nc.dram_tensor("y0", [BSH, C], f32, kind="ExternalOutput")
    y1o = nc.dram_tensor("y1", [BSH, D * C], f32, kind="ExternalOutput")

    groups = [(s, z, bias_head) for (s, z) in _GROUPS_HEAD]
    groups += [(s, z, True) for (s, z) in _GROUPS_RES]

    with tile.TileContext(nc) as tc:
        with (
            tc.tile_pool(name="wpool", bufs=1) as wpool,
            tc.tile_pool(name="xpool", bufs=3) as xpool,
            tc.tile_pool(name="ypool", bufs=2) as ypool,
            tc.tile_pool(name="spool", bufs=3) as spool,
            tc.tile_pool(name="psum", bufs=8, space="PSUM") as pp,
        ):
            # Weights resident in SBUF: one [128, NCOL] tile per k-chunk.
            # Big loads alternate between the two HWDGE queues (SP / ACT).
            dmae = [nc.sync, nc.scalar]
            wtiles = []
            for kc in range(KC):
                wt = wpool.tile([P, NCOL], f32, tag=f"w{kc}")
                dmae[kc % 2].dma_start(out=wt, in_=W[kc * P : (kc + 1) * P, :])
                wtiles.append(wt)
            brow_sb = wpool.tile([1, NCOL], f32, tag="brow")
            nc.sync.dma_start(out=brow_sb, in_=brow[:, :])
            ones_sb = wpool.tile([1, P], f32, tag="ones")
            nc.vector.memset(ones_sb, 1.0)

            xt2 = None
            for bt in range(NTILES):
                t2, bt2 = divmod(bt, 2)
                if bt2 == 0:
                    xt2 = xpool.tile([P, 2 * KC * P], f32, tag="xt")
                    dmae[t2 % 2].dma_start(out=xt2, in_=xP[t2, :, :])
                yt = ypool.tile([P, NCOL], f32, tag="yt")

                # k-chunk OUTER so one stationary x chunk serves all column
                # groups back-to-back (LDWEIGHTS between matmuls otherwise
                # serializes each matmul's full fill+drain latency).
                pgs = [
                    pp.tile([P, 512], f32, tag="pg", name=f"pg{gi}")
                    for gi in range(len(groups))
                ]
                for gi, (g0, gsz, has_b) in enumerate(groups):
                    if has_b:
                        nc.tensor.matmul(
                            pgs[gi][:, :gsz],
                            ones_sb,
                            brow_sb[:, g0 : g0 + gsz],
                            start=True,
                            stop=False,
                        )
                for kc in range(KC):
                    xoff = (bt2 * KC + kc) * P
                    for gi, (g0, gsz, has_b) in enumerate(groups):
                        nc.tensor.matmul(
                            pgs[gi][:, :gsz],
                            xt2[:, xoff : xoff + P],
                            wtiles[kc][:, g0 : g0 + gsz],
                            start=(not has_b) and kc == 0,
                            stop=kc == KC - 1,
                        )
                for gi, (g0, gsz, has_b) in enumerate(groups):
                    nc.scalar.copy(out=yt[:, g0 : g0 + gsz], in_=pgs[gi][:, :gsz])

                # --- argmax / one-hot selection (vector engine) ---
                maxc = spool.tile([P, C], f32, tag="maxc")
                nc.vector.reduce_max(
                    out=maxc,
                    in_=yt[:, BIN_OFF : BIN_OFF + CK].rearrange(
                        "p (c k) -> p c k", k=K
                    ),
                    axis=X,
                )
                mask = spool.tile([P, CK], f32, tag="mask")
                for c in range(C):
                    nc.vector.tensor_scalar(
                        out=mask[:, c * K : (c + 1) * K],
                        in0=yt[:, BIN_OFF + c * K : BIN_OFF + (c + 1) * K],
                        scalar1=maxc[:, c : c + 1],
                        scalar2=None,
                        op0=mybir.AluOpType.is_ge,
                    )
                y1t = spool.tile([P, D * C], f32, tag="y1t")
                for d in range(D):
                    tmp = spool.tile([P, CK], f32, tag="tmp")
                    nc.vector.tensor_mul(
                        out=tmp,
                        in0=mask,
                        in1=yt[:, RES_OFF + d * CK : RES_OFF + (d + 1) * CK],
                    )
                    nc.vector.reduce_sum(
                        out=y1t[:, d * C : (d + 1) * C],
                        in_=tmp.rearrange("p (c k) -> p c k", k=K),
                        axis=X,
                    )

                nc.sync.dma_start(
                    out=y0o[bt * P : (bt + 1) * P, :], in_=yt[:, 0:C]
                )
                nc.sync.dma_start(
                    out=y1o[bt * P : (bt + 1) * P, :], in_=y1t
                )

    return nc


def _get_nc(bias_head):
    key = bool(bias_head)
    if key not in _CACHE:
        _CACHE[key] = _build(key)
    return _CACHE[key]


def kernel(x, Wfc, bfc, Wbin, bbin, Wres, bres, centers):
    from concourse.bass_utils import run_bass_kernel_spmd

    x = np.asarray(x, dtype=np.float32)
    Wfc = np.asarray(Wfc, dtype=np.float32)
    bfc = np.asarray(bfc, dtype=np.float32)
    Wbin = np.asarray(Wbin, dtype=np.float32)
    bbin = np.asarray(bbin, dtype=np.float32)
    Wres = np.asarray(Wres, dtype=np.float32)
    bres = np.asarray(bres, dtype=np.float32)
    centers = np.asarray(centers, dtype=np.float32)

    # Fused weight matrix [N0, NCOL]: [Wfc | Wbin(c,k) | Wres(d,c,k)].
    W_cat = np.empty((N0, NCOL), dtype=np.float32)
    W_cat[:, :C] = Wfc
    W_cat[:, BIN_OFF:RES_OFF] = Wbin.transpose(2, 0, 1).reshape(N0, CK)
    W_cat[:, RES_OFF:] = Wres.transpose(2, 3, 0, 1).reshape(N0, RES)

    # Bias row; centers fold into the residual bias (selection distributes
    # over the sum).
    brow = np.zeros((1, NCOL), dtype=np.float32)
    brow[0, :C] = bfc
    brow[0, BIN_OFF:RES_OFF] = bbin.reshape(CK)
    brow[0, RES_OFF:] = (bres + centers[None, :, :]).transpose(2, 0, 1).reshape(RES)

    bias_head = bool(np.any(bfc) or np.any(bbin))
    nc = _get_nc(bias_head)

    # Pre-tile x per core: xP[t2, p, bt2, kc, b] = x_shard[t2*256 + bt2*128 + b,
    # kc*128 + p] -> contiguous 8KB per SBUF partition row per load.
    xv = x.reshape(NCORES, NTILES // 2, 2, P, KC, P)  # (core, t2, bt2, b, kc, p)
    in_maps = []
    for i in range(NCORES):
        xPi = np.ascontiguousarray(xv[i].transpose(0, 4, 1, 3, 2)).reshape(
            NTILES // 2, P, 2 * KC * P
        )
        in_maps.append({"xP": xPi, "W": W_cat, "brow": brow})

    kwargs = {}
    if os.environ.get("BASS_KERNEL_TRACE") == "1":
        kwargs["trace"] = True
        td = os.environ.get("BASS_KERNEL_TRACE_DIR")
        if td:
            os.makedirs(td, exist_ok=True)
            kwargs["tmpdir"] = td

    res = run_bass_kernel_spmd(nc, in_maps, core_ids=list(range(NCORES)), **kwargs)
    if kwargs.get("trace"):
        kernel.last_exec_time_ns = res.exec_time_ns

    y0 = np.concatenate([res.results[i]["y0"] for i in range(NCORES)], axis=0)
    y1 = np.concatenate([res.results[i]["y1"] for i in range(NCORES)], axis=0)
    return y0, y1.reshape(B, D, C)


# revision 10
# speedup vs baseline: 2.3578x; 2.3578x over previous
"""Trainium2 Bass kernel for nn_JointCatPoseModel (moe_routing).

Computation (reference):
    y0   = x @ Wfc + bfc                         [B, C]
    ybin = einsum('bn,ckn->bck', x, Wbin) + bbin [B, C, K]
    ind  = argmax_k ybin                         [B, C]
    yres = (einsum('bn,cknd->bckd', x, Wres) + bres)[b, c, ind, d]
    y1   = (centers[ind] + yres).transpose(0, 2, 1)   [B, D, C]

Strategy: data-parallel over batch across 8 NeuronCores (2048 rows each,
weights replicated, no collectives).  Per core, per 128-row batch tile,
one fused GEMM in 7 PSUM column groups (contraction split into 8 chunks
of 128):

  - ybin (768 cols) in native fp32: the argmax over clusters is
    bit-sensitive (top-2 logit gaps go down to 2e-6), bf16 would flip
    ~1% of decisions.  fp32 matmul runs as 2 hi/lo passes on the PE.
  - y0 + yres (2316 cols) in bf16: these only face a direct-output
    error (~2e-3 relative), no discrete decisions; bf16 matmul is ~4x
    cheaper than fp32.
  - biases + cluster centers (folded into the residual bias since the
    one-hot selection distributes over +) are added in exact fp32 by the
    vector engine during the PSUM->SBUF move.

Selection runs on the vector engine: per-class reduce_max -> is_ge
one-hot mask -> masked multiply + per-class reduce_sum, writing the
[d][c]-ordered y1 output directly.
"""

import os

import numpy as np

B, N0, C, K, D = 16384, 1024, 12, 64, 3
NCORES = 8
BSH = B // NCORES          # 2048 rows per core
P = 128                    # partitions / batch tile rows
NTILES = BSH // P          # 16 batch tiles per core
KC = N0 // P               # 8 contraction chunks
CK = C * K                 # 768 bin columns
RES = C * K * D            # 2304 residual columns
NCOL = CK + C + RES        # 3084 fused columns in SBUF Y tile
# Y tile layout: [ ybin 0:768 | y0 768:780 | yres 780:3084 ]
BIN_OFF = 0
Y0_OFF = CK
RES_OFF = CK + C
NBF = C + RES              # 2316 bf16 GEMM columns ([Wfc | Wres(d,c,k)])

# PSUM column groups over the Y tile: (yt_start, size).
_GROUPS_F32 = [(0, 512), (512, 256)]                       # ybin
_GROUPS_BF = [(768, 512), (1280, 512), (1792, 512), (2304, 512), (2816, 268)]

_CACHE = {}


def _patch_tail_drain():
    """Cap sync waits at one per instruction.

    The walrus build in this container rejects instructions carrying more
    than one sync wait ("Too many sync wait commands"), but Tile's
    scheduler freely attaches several.  Semantically, waits attached to an
    instruction execute before it; moving all but one wait onto preceding
    same-engine NOPs is equivalent (the engine blocks on the NOPs first).
    Also splits the tail drain, which Tile builds outside the commit path.
    """
    import concourse.mybir as mybir
    from concourse.tile import ScopedClock, TileContext

    if getattr(TileContext, "_drain_split_patched", False):
        return

    _orig_commit = TileContext._commit_instruction

    def _commit_instruction(self, inst, lazy_reg_writes=True):
        si = inst.sync_info
        if si is not None and len(si.on_wait) > 1:
            waits = list(si.on_wait)
            inst.sync_info = mybir.SyncInfo(
                on_wait=waits[-1:], on_update=list(si.on_update)
            )
            for w in waits[:-1]:
                nop = mybir.InstNoOp(
                    name=self.nc.get_next_instruction_name(),
                    text_hint="wait_split",
                    bass_nofuse=True,
                    engine=inst.engine,
                    sync_info=mybir.SyncInfo(on_wait=[w], on_update=[]),
                )
                _orig_commit(self, nop, lazy_reg_writes=False)
        return _orig_commit(self, inst, lazy_reg_writes=lazy_reg_writes)

    def _drain_and_barrier(self, tick_clock, wait_clock):
        nc = self.nc
        drain_inst = nc.sync.drain()
        wait_clock.add_sem_waits(
            drain_inst.ins, ScopedClock({None: tick_clock.global_clock})
        )
        si = drain_inst.ins.sync_info
        waits = list(si.on_wait) if si else []
        if len(waits) > 1:
            drain_inst.ins.sync_info = mybir.SyncInfo(
                on_wait=waits[:1], on_update=list(si.on_update)
            )
            for w in waits[1:]:
                d2 = nc.sync.drain()
                d2.ins.sync_info = mybir.SyncInfo(on_wait=[w], on_update=[])
        nc.all_engine_barrier()
        popped = nc._tile_sem_poison_stack.pop()
        assert popped is self._sem_poison
        nc.clear_and_free_semaphores(list(self.sems.allocated().values()))
        nc.all_engine_barrier()

    TileContext._commit_instruction = _commit_instruction
    TileContext._drain_and_barrier = _drain_and_barrier
    TileContext._drain_split_patched = True


def _build(bias_bin):
    """Build the per-core Bass module.  bias_bin: add a bias to the ybin
    region too (needed only when bbin is nonzero; it is zero in the spec)."""
    import concourse.bass as bass
    import concourse.mybir as mybir
    import concourse.tile as tile

    _patch_tail_drain()
    f32 = mybir.dt.float32
    bf16 = mybir.dt.bfloat16
    X = mybir.AxisListType.X

    nc = bass.Bass(trn_type="TRN2")
    # x arrives pre-tiled: xP[t2, p, (bt2 kc b)] = x[t2*256 + bt2*128 + b,
    # kc*128 + p] -> each load is [128 partitions x 8KB contiguous].
    xP = nc.dram_tensor("xP", [NTILES // 2, P, 2 * KC * P], f32, kind="ExternalInput")
    xPh = nc.dram_tensor(
        "xPh", [NTILES // 2, P, 2 * KC * P], bf16, kind="ExternalInput"
    )
    Wf = nc.dram_tensor("Wf", [N0, CK], f32, kind="ExternalInput")      # Wbin(c,k)
    Wh = nc.dram_tensor("Wh", [N0, NBF], bf16, kind="ExternalInput")    # Wfc|Wres(dck)
    # replicated bias rows for the bf16 section (and ybin if bias_bin)
    bias_b = nc.dram_tensor("bias_b", [P, NBF], f32, kind="ExternalInput")
    bias_n = (
        nc.dram_tensor("bias_n", [P, CK], f32, kind="ExternalInput")
        if bias_bin
        else None
    )
    y0o = nc.dram_tensor("y0", [BSH, C], f32, kind="ExternalOutput")
    y1o = nc.dram_tensor("y1", [BSH, D * C], f32, kind="ExternalOutput")

    with tile.TileContext(nc) as tc:
        with (
            tc.tile_pool(name="wpool", bufs=1) as wpool,
            tc.tile_pool(name="xpool", bufs=2) as xpool,
            tc.tile_pool(name="ypool", bufs=2) as ypool,
            tc.tile_pool(name="spool", bufs=3) as spool,
            tc.tile_pool(name="psum", bufs=8, space="PSUM") as pp,
        ):
            # Weights resident in SBUF, one tile per k-chunk; big loads
            # alternate between the two HWDGE queues (SP / ACT).
            dmae = [nc.sync, nc.scalar]
            wf_tiles = []
            wh_tiles = []
            for kc in range(KC):
                wf = wpool.tile([P, CK], f32, tag=f"wf{kc}", name=f"wf{kc}")
                dmae[kc % 2].dma_start(out=wf, in_=Wf[kc * P : (kc + 1) * P, :])
                wf_tiles.append(wf)
                wh = wpool.tile([P, NBF], bf16, tag=f"wh{kc}", name=f"wh{kc}")
                dmae[(kc + 1) % 2].dma_start(out=wh, in_=Wh[kc * P : (kc + 1) * P, :])
                wh_tiles.append(wh)
            bias_sb = wpool.tile([P, NBF], f32, tag="bias")
            nc.sync.dma_start(out=bias_sb, in_=bias_b[:, :])
            if bias_bin:
                bias_nsb = wpool.tile([P, CK], f32, tag="bias_n")
                nc.scalar.dma_start(out=bias_nsb, in_=bias_n[:, :])

            xt2 = xt2h = None
            for bt in range(NTILES):
                t2, bt2 = divmod(bt, 2)
                if bt2 == 0:
                    xt2 = xpool.tile([P, 2 * KC * P], f32, tag="xt")
                    dmae[t2 % 2].dma_start(out=xt2, in_=xP[t2, :, :])
                    xt2h = xpool.tile([P, 2 * KC * P], bf16, tag="xth")
                    dmae[(t2 + 1) % 2].dma_start(out=xt2h, in_=xPh[t2, :, :])
                yt = ypool.tile([P, NCOL], f32, tag="yt")

                # k-chunk OUTER so one stationary x chunk serves all column
                # groups back-to-back.
                pgs = [
                    pp.tile([P, 512], f32, tag="pg", name=f"pg{gi}")
                    for gi in range(len(_GROUPS_F32) + len(_GROUPS_BF))
                ]
                for kc in range(KC):
                    xoff = (bt2 * KC + kc) * P
                    for gi, (g0, gsz) in enumerate(_GROUPS_F32):
                        nc.tensor.matmul(
                            pgs[gi][:, :gsz],
                            xt2[:, xoff : xoff + P],
                            wf_tiles[kc][:, g0 : g0 + gsz],
                            start=kc == 0,
                            stop=kc == KC - 1,
                        )
                    for gj, (g0, gsz) in enumerate(_GROUPS_BF):
                        gi = len(_GROUPS_F32) + gj
                        w0 = g0 - CK
                        nc.tensor.matmul(
                            pgs[gi][:, :gsz],
                            xt2h[:, xoff : xoff + P],
                            wh_tiles[kc][:, w0 : w0 + gsz],
                            start=kc == 0,
                            stop=kc == KC - 1,
                        )

                # PSUM -> SBUF: ACT copies the fp32 ybin groups (plus bias
                # on the rare nonzero-bbin path via DVE); DVE adds the fp32
                # bias to the bf16 groups.
                for gi, (g0, gsz) in enumerate(_GROUPS_F32):
                    if bias_bin:
                        nc.vector.tensor_add(
                            out=yt[:, g0 : g0 + gsz],
                            in0=pgs[gi][:, :gsz],
                            in1=bias_nsb[:, g0 : g0 + gsz],
                        )
                    else:
                        nc.scalar.copy(out=yt[:, g0 : g0 + gsz], in_=pgs[gi][:, :gsz])
                for gj, (g0, gsz) in enumerate(_GROUPS_BF):
                    gi = len(_GROUPS_F32) + gj
                    w0 = g0 - CK
                    nc.vector.tensor_add(
                        out=yt[:, g0 : g0 + gsz],
                        in0=pgs[gi][:, :gsz],
                        in1=bias_sb[:, w0 : w0 + gsz],
                    )

                # --- argmax / one-hot selection (vector engine) ---
                maxc = spool.tile([P, C], f32, tag="maxc")
                nc.vector.reduce_max(
                    out=maxc,
                    in_=yt[:, BIN_OFF : BIN_OFF + CK].rearrange(
                        "p (c k) -> p c k", k=K
                    ),
                    axis=X,
                )
                mask = spool.tile([P, CK], f32, tag="mask")
                for c in range(C):
                    nc.vector.tensor_scalar(
                        out=mask[:, c * K : (c + 1) * K],
                        in0=yt[:, BIN_OFF + c * K : BIN_OFF + (c + 1) * K],
                        scalar1=maxc[:, c : c + 1],
                        scalar2=None,
                        op0=mybir.AluOpType.is_ge,
                    )
                y1t = spool.tile([P, D * C], f32, tag="y1t")
                for d in range(D):
                    tmp = spool.tile([P, CK], f32, tag="tmp")
                    nc.vector.tensor_mul(
                        out=tmp,
                        in0=mask,
                        in1=yt[:, RES_OFF + d * CK : RES_OFF + (d + 1) * CK],
                    )
                    nc.vector.reduce_sum(
                        out=y1t[:, d * C : (d + 1) * C],
                        in_=tmp.rearrange("p (c k) -> p c k", k=K),
                        axis=X,
                    )

                nc.sync.dma_start(
                    out=y0o[bt * P : (bt + 1) * P, :],
                    in_=yt[:, Y0_OFF : Y0_OFF + C],
                )
                nc.sync.dma_start(
                    out=y1o[bt * P : (bt + 1) * P, :], in_=y1t
                )

    return nc


def _get_nc(bias_bin):
    key = bool(bias_bin)
    if key not in _CACHE:
        _CACHE[key] = _build(key)
    return _CACHE[key]


def kernel(x, Wfc, bfc, Wbin, bbin, Wres, bres, centers):
    import ml_dtypes

    from concourse.bass_utils import run_bass_kernel_spmd

    x = np.asarray(x, dtype=np.float32)
    Wfc = np.asarray(Wfc, dtype=np.float32)
    bfc = np.asarray(bfc, dtype=np.float32)
    Wbin = np.asarray(Wbin, dtype=np.float32)
    bbin = np.asarray(bbin, dtype=np.float32)
    Wres = np.asarray(Wres, dtype=np.float32)
    bres = np.asarray(bres, dtype=np.float32)
    centers = np.asarray(centers, dtype=np.float32)

    bf = ml_dtypes.bfloat16

    # fp32 GEMM weights: Wbin as [n, (c k)]
    W_f32 = np.ascontiguousarray(Wbin.transpose(2, 0, 1).reshape(N0, CK))
    # bf16 GEMM weights: [Wfc | Wres(d,c,k)]
    W_bf = np.empty((N0, NBF), dtype=bf)
    W_bf[:, :C] = Wfc.astype(bf)
    W_bf[:, C:] = Wres.transpose(2, 3, 0, 1).reshape(N0, RES).astype(bf)

    # fp32 bias over the bf16 section [y0 | yres]; centers fold into the
    # residual bias (the one-hot selection distributes over +).
    bias_row = np.empty((NBF,), dtype=np.float32)
    bias_row[:C] = bfc
    bias_row[C:] = (bres + centers[None, :, :]).transpose(2, 0, 1).reshape(RES)
    bias_b = np.ascontiguousarray(np.broadcast_to(bias_row, (P, NBF)))

    bias_bin = bool(np.any(bbin))
    nc = _get_nc(bias_bin)

    # Pre-tile x per core: xP[t2, p, (bt2 kc b)] = x_shard[t2*256 + bt2*128
    # + b, kc*128 + p] -> contiguous per-partition rows per load.
    xv = x.reshape(NCORES, NTILES // 2, 2, P, KC, P)  # (core, t2, bt2, b, kc, p)
    in_maps = []
    for i in range(NCORES):
        xPi = np.ascontiguousarray(xv[i].transpose(0, 4, 1, 3, 2)).reshape(
            NTILES // 2, P, 2 * KC * P
        )
        m = {
            "xP": xPi,
            "xPh": xPi.astype(bf),
            "Wf": W_f32,
            "Wh": W_bf,
            "bias_b": bias_b,
        }
        if bias_bin:
            m["bias_n"] = np.ascontiguousarray(
                np.broadcast_to(bbin.reshape(CK), (P, CK))
            ).astype(np.float32)
        in_maps.append(m)

    kwargs = {}
    if os.environ.get("BASS_KERNEL_TRACE") == "1":
        kwargs["trace"] = True
        td = os.environ.get("BASS_KERNEL_TRACE_DIR")
        if td:
            os.makedirs(td, exist_ok=True)
            kwargs["tmpdir"] = td

    res = run_bass_kernel_spmd(nc, in_maps, core_ids=list(range(NCORES)), **kwargs)
    if kwargs.get("trace"):
        kernel.last_exec_time_ns = res.exec_time_ns

    y0 = np.concatenate([res.results[i]["y0"] for i in range(NCORES)], axis=0)
    y1 = np.concatenate([res.results[i]["y1"] for i in range(NCORES)], axis=0)
    return y0, y1.reshape(B, D, C)
